# revision 17
# baseline (speedup 1.0000x reference)
"""Trainium2 Bass kernel for nn_Attention_39608188404100 (v4.1).

Windowed-attention block (ViT-style, N=197 tokens) with SSF affines, relative
position bias, DCF head mixing, and output projection.

Strategy: pure data-parallel over batch across 8 NeuronCores (B=64 -> 8/core).
All weights replicated; no collectives. bf16 on the PE, fp32 PSUM.

Per core (BL=8 batches): each batch's 197 tokens are padded to 200 positions
and PERMUTED on host: position p = c*100 + ml*10 + g holds token
m = c*100 + g*10 + ml (c = chunk, 2x100). The 3 dummy positions get zero
x-columns and exp(-40) relative-bias multipliers, so they vanish in softmax;
dummy query columns are dropped on host after download.

v4.1 structure (no DRAM scratch; batch-PAIR pipeline to amortize the ~0.8us
per-DMA-trigger cost on the sync queue):
  - Stage 1: all 12 q/k channel-tile GEMMs back to back; uploads split so the
    first matmul starts after ~1MB.
  - Pair loop (pr = 0..3, batches 2pr/2pr+1): per pair, scores/exp/relb-mult/
    den(ones-matmul)/normalize build et2 [100, (h b2 cn)] (double-buffered);
    one-hop SBUF->SBUF shuffle (10 j-split DMAs, 1600B runs) to
    mxin2 [(wgi h), (j b2 cn)]; block-diag mix matmul; second one-hop shuffle
    (10 j-split DMAs) back to key-partition a2 (ALIASES et2); AV; 400-token
    projection (4 chunks) per pair.
  - All shuffle DMA access patterns are partition-contiguous on BOTH sides
    (the [120 x run] side is expressed as a plain 2D slice).
  - Drain work is spread over three engines: exp + AV drains on Scalar,
    norm/recip/mix drains on Vector, relb-mult + v/proj drains on GpSimd.
  - The v-bias is folded into the proj bias host-side (rows of the
    normalized+mixed attention sum to m_k = 1 + sum_h coeff[k,h]).

Env:
  BASS_KERNEL_PROFILE=1  capture neuron-profile (exec_time_ns) on the run.
"""
import os
import sys

sys.path.insert(0, "/opt/trn_rl_repo")

import numpy as np
import ml_dtypes

import concourse.bass as bass
import concourse.tile as tile
from concourse import bacc, mybir

BF16 = mybir.dt.bfloat16
F32 = mybir.dt.float32
AF = mybir.ActivationFunctionType
ALU = mybir.AluOpType

B, N, C, H, DH = 64, 197, 768, 12, 64
NCORES = 8
BL = B // NCORES          # 8 batches per core
NPR = BL // 2             # 4 batch pairs per core
P2 = 200                  # padded positions per batch
T2 = BL * P2              # 1600 positions per core
SCALE = DH ** -0.5
KT = 6                    # contraction tiles of 128 over C=768
QKM = 12                  # 128-wide M tiles over 1536 q/k channels
TOKC = [(0, 500), (500, 500), (1000, 500), (1500, 100)]  # 100-aligned chunks
TOKBASE = [0, 3000, 6000, 9000]
DUMMY_BIAS = -40.0
E2 = 2 * 2 * P2           # 800: per-head (b2, c, n) block in et2/a2
ECOLS2 = H * E2           # 9600: et2 cols
MCOLS2 = 10 * E2          # 8000: mxin2/mxo2 cols

_COMPILED = {}


def _build_graph():
    # detect_race_conditions=False: the sim race-detector's shadow model
    # linearizes multi-dim DMA APs as byte offsets and reports false overlaps
    # between distinct tiles; value semantics were validated in CoreSim and
    # against hardware.
    nc = bacc.Bacc(
        "TRN2", target_bir_lowering=False, debug=False,
        detect_race_conditions=False,
    )

    xT_d = nc.dram_tensor("xT", [128, KT * T2], BF16, kind="ExternalInput")
    wqk_d = nc.dram_tensor("wqk", [128, QKM * 768], BF16, kind="ExternalInput")
    wv_d = nc.dram_tensor("wv", [128, KT * 768], BF16, kind="ExternalInput")
    wp_d = nc.dram_tensor("wp", [128, KT * 768], BF16, kind="ExternalInput")
    relb_d = nc.dram_tensor("relb", [100, H * 2 * P2], BF16, kind="ExternalInput")
    mix_d = nc.dram_tensor("mixblk", [120, 120], BF16, kind="ExternalInput")
    bqk_d = nc.dram_tensor("bqk", [128, QKM], F32, kind="ExternalInput")
    bp_d = nc.dram_tensor("bp", [128, 768], BF16, kind="ExternalInput")
    out_d = nc.dram_tensor("out", [T2, 768], BF16, kind="ExternalOutput")

    with tile.TileContext(nc) as tc:
        with (
            tc.tile_pool(name="const", bufs=1) as cpool,
            tc.tile_pool(name="dch", bufs=2) as dpool,
            tc.tile_pool(name="osb", bufs=2) as opool,
            tc.tile_pool(name="psA", bufs=2, space=bass.MemorySpace.PSUM) as psA,
            tc.tile_pool(name="psSC", bufs=2, space=bass.MemorySpace.PSUM) as psSC,
            tc.tile_pool(name="psMX", bufs=2, space=bass.MemorySpace.PSUM) as psMX,
            tc.tile_pool(name="psDV", bufs=2, space=bass.MemorySpace.PSUM) as psDV,
        ):
            # ---- persistent tiles ----
            xT = cpool.tile([128, KT * T2], BF16)
            qk_t = [cpool.tile([128, T2], BF16, name=f"qk{mt}")
                    for mt in range(QKM)]
            relb = cpool.tile([100, H * 2 * P2], BF16)
            wv = cpool.tile([128, KT * 768], BF16)
            wp = cpool.tile([128, KT * 768], BF16)
            mixblk = cpool.tile([120, 120], BF16)
            bqk = cpool.tile([128, QKM], F32)
            bp = cpool.tile([128, 768], BF16)
            ones_den = cpool.tile([128, 128], BF16)
            et2 = [cpool.tile([128, ECOLS2], BF16, name=f"et2_{s}")
                   for s in range(2)]                 # double-buffered; a2 alias
            mxin2 = cpool.tile([120, MCOLS2], BF16)
            mxo2 = cpool.tile([120, MCOLS2], BF16)
            aoT2 = [cpool.tile([128, KT * 2 * P2], BF16, name=f"aoT2_{s}")
                    for s in range(2)]                # out^T per pair
            vring = [cpool.tile([100, 2 * 768], BF16, name=f"v{s}")
                     for s in range(4)]

            nc.vector.memset(ones_den[:], 1.0)
            nc.vector.memset(et2[0][:], 0.0)
            nc.vector.memset(et2[1][:], 0.0)
            nc.vector.memset(mxin2[:], 0.0)
            nc.vector.memset(mxo2[:], 0.0)

            def xt_slice(t0, kt, w):
                """xT AP for tokens [t0, t0+w) at contraction tile kt."""
                ci = min(t0 // 500, 3)
                base, off, nsz_c = TOKBASE[ci], t0 - TOKC[ci][0], TOKC[ci][1]
                col = base + kt * nsz_c + off
                return xT[:, col: col + w]

            # ---- stage 1: q/k GEMMs, uploads overlapped ----
            with tc.tile_pool(name="wqk", bufs=1) as wpool:
                wqk = wpool.tile([128, QKM * 768], BF16)
                # critical-path uploads first
                nc.sync.dma_start(wqk[:, 0:768], wqk_d[:, 0:768])
                nc.sync.dma_start(xT[:, 0:3000], xT_d[:, 0:3000])
                nc.sync.dma_start(bqk[:], bqk_d[:])
                nc.sync.dma_start(wqk[:, 768:], wqk_d[:, 768:])
                nc.sync.dma_start(xT[:, 3000:], xT_d[:, 3000:])
                nc.sync.dma_start(relb[:], relb_d[:])
                nc.sync.dma_start(wv[:], wv_d[:])
                nc.sync.dma_start(mixblk[:], mix_d[:])
                nc.sync.dma_start(wp[:], wp_d[:])
                nc.sync.dma_start(bp[:], bp_d[:])

                for mt in range(QKM):
                    for ci, (n0, nsz) in enumerate(TOKC):
                        ps = psA.tile([128, 512], F32, tag="a")
                        for kt in range(KT):
                            nc.tensor.matmul(
                                ps[:, 0:nsz],
                                wqk[:, mt * 768 + kt * 128: mt * 768 + (kt + 1) * 128],
                                xT[:, TOKBASE[ci] + kt * nsz: TOKBASE[ci] + kt * nsz + nsz],
                                start=(kt == 0),
                                stop=(kt == KT - 1),
                            )
                        nc.scalar.activation(
                            qk_t[mt][:, n0:n0 + nsz],
                            ps[:, 0:nsz],
                            AF.Identity,
                            bias=bqk[:, mt:mt + 1],
                            scale=1.0,
                        )

            # ---- pair-loop pieces ----
            def vproj(b):
                """v for batch b -> vring[b % 4] (no bias: folded into bp)."""
                vt = vring[b % 4]
                for c in range(2):
                    t0 = b * P2 + c * 100
                    for n0, nsz in ((0, 512), (512, 256)):
                        ps = psA.tile([128, 512], F32, tag="a")
                        for kt in range(KT):
                            nc.tensor.matmul(
                                ps[0:100, 0:nsz],
                                xt_slice(t0, kt, 100),
                                wv[:, kt * 768 + n0: kt * 768 + n0 + nsz],
                                start=(kt == 0),
                                stop=(kt == KT - 1),
                            )
                        nc.scalar.copy(
                            vt[0:100, c * 768 + n0: c * 768 + n0 + nsz],
                            ps[0:100, 0:nsz],
                        )

            def sdn(b):
                """scores -> exp -> *relb -> den -> normalize for batch b
                into the (h, b2, c, n) columns of et2[(b//2) % 2]."""
                b2 = b % 2
                et = et2[(b // 2) % 2][0:100, :]
                # [p, h, b2, c, n] and [p, h, b2, (c n)] views
                etv = et.rearrange("p (h b2 c n) -> p h b2 c n",
                                   h=H, b2=2, c=2, n=P2)
                etb = et.rearrange("p (h b2 cn) -> p h b2 cn",
                                   h=H, b2=2, cn=2 * P2)
                relv = relb[:].rearrange("p (h cn) -> p h cn", h=H, cn=2 * P2)

                def sc(tq):
                    for hh in range(4):
                        h = 4 * tq + hh
                        prow = (h % 2) * 64
                        qt = qk_t[h // 2]
                        kt_ = qk_t[6 + h // 2]
                        ps1 = psSC.tile([128, 512], F32, tag="sc")
                        nc.tensor.matmul(
                            ps1[0:100, 0:P2],
                            kt_[prow:prow + 64, b * P2: b * P2 + 100],
                            qt[prow:prow + 64, b * P2: b * P2 + P2],
                            start=True, stop=True,
                        )
                        nc.tensor.matmul(
                            ps1[0:100, P2:2 * P2],
                            kt_[prow:prow + 64, b * P2 + 100: b * P2 + 200],
                            qt[prow:prow + 64, b * P2: b * P2 + P2],
                            start=True, stop=True,
                        )
                        nc.scalar.activation(
                            etb[:, h, b2, :],
                            ps1[0:100, 0:2 * P2], AF.Exp,
                        )
                    nc.gpsimd.tensor_tensor(
                        etb[:, 4 * tq:4 * tq + 4, b2, :],
                        etb[:, 4 * tq:4 * tq + 4, b2, :],
                        relv[:, 4 * tq:4 * tq + 4, :],
                        ALU.mult,
                    )

                def den(tq):
                    dch = dpool.tile([100, 800], F32, tag="dch")
                    for dd in range(2):
                        psd = psDV.tile([128, 512], F32, tag="dv")
                        nc.tensor.matmul(
                            psd[0:100, 0:400],
                            ones_den[0:100, 0:100],
                            etv[:, 4 * tq + 2 * dd: 4 * tq + 2 * dd + 2, b2, 0, :],
                            start=True, stop=False,
                        )
                        nc.tensor.matmul(
                            psd[0:100, 0:400],
                            ones_den[0:100, 0:100],
                            etv[:, 4 * tq + 2 * dd: 4 * tq + 2 * dd + 2, b2, 1, :],
                            start=False, stop=True,
                        )
                        nc.vector.reciprocal_approx_fast(
                            dch[:, dd * 400:(dd + 1) * 400],
                            psd[0:100, 0:400],
                        )
                    dv4 = dch[:].rearrange("p (h n) -> p h n", h=4)
                    for c in range(2):
                        nc.gpsimd.tensor_tensor(
                            etv[:, 4 * tq:4 * tq + 4, b2, c, :],
                            etv[:, 4 * tq:4 * tq + 4, b2, c, :],
                            dv4, ALU.mult,
                        )

                sc(0)
                sc(1)
                den(0)
                sc(2)
                den(1)
                den(2)

            def sdn2(pr):
                sdn(2 * pr)
                sdn(2 * pr + 1)

            def hop2(pr):
                """et2 [(j wgi), (h b2 cn)] -> mxin2 [(wgi h), (j b2 cn)].
                j-split; both sides partition-contiguous; 1600B runs."""
                et_v = et2[pr % 2][0:100, :].rearrange(
                    "(j wgi) f -> j wgi f", j=10)
                for j in range(10):
                    nc.sync.dma_start(
                        mxin2[:, j * E2: (j + 1) * E2], et_v[j])

            def mix(pr):
                for i, o in enumerate(range(0, MCOLS2, 500)):
                    psm = psMX.tile([128, 512], F32, tag="mx")
                    nc.tensor.matmul(
                        psm[0:120, 0:500], mixblk[:],
                        mxin2[:, o:o + 500],
                        start=True, stop=True,
                    )
                    nc.vector.tensor_scalar_add(
                        mxo2[:, o:o + 500], psm[0:120, 0:500], 0.0
                    )

            def hop3(pr):
                """mxo2 [(wgi k), (j b2 cn)] -> a2 [(j wgi), (k b2 cn)].
                j-split; src is a plain [120, E2] slice; 1600B runs."""
                a2_v = et2[pr % 2][0:100, :].rearrange(
                    "(j wgi) (k f) -> j wgi k f", wgi=10, f=E2)
                for j in range(10):
                    nc.sync.dma_start(
                        a2_v[j], mxo2[:, j * E2: (j + 1) * E2])

            def av(b):
                b2 = b % 2
                a2 = et2[(b // 2) % 2][0:100, :]
                ao = aoT2[(b // 2) % 2]
                vt = vring[b % 4]
                for jj in range(H // 2):
                    pv = psDV.tile([128, 512], F32, tag="dv")
                    for sub in range(2):
                        k = 2 * jj + sub
                        rows = pv[sub * 64: sub * 64 + 64, 0:P2]
                        tp = (0, sub * 64)
                        for c in range(2):
                            nc.tensor.matmul(
                                rows,
                                vt[0:100, c * 768 + k * 64: c * 768 + (k + 1) * 64],
                                a2[0:100, k * E2 + b2 * 2 * P2 + c * P2:
                                   k * E2 + b2 * 2 * P2 + c * P2 + P2],
                                start=(c == 0),
                                stop=(c == 1),
                                tile_position=tp,
                            )
                    nc.scalar.copy(
                        ao[:, jj * 2 * P2 + b2 * P2: jj * 2 * P2 + (b2 + 1) * P2],
                        pv[:, 0:P2],
                    )

            def proj2(pr):
                """output projection for the pair's 400 tokens, 4 chunks."""
                ao = aoT2[pr % 2]
                for t0, tsz in ((0, 128), (128, 128), (256, 128), (384, 16)):
                    osb = opool.tile([128, 768], BF16, tag="osb")
                    for n0, nsz in ((0, 512), (512, 256)):
                        pp = psA.tile([128, 512], F32, tag="a")
                        for kt in range(KT):
                            nc.tensor.matmul(
                                pp[0:tsz, 0:nsz],
                                ao[:, kt * 2 * P2 + t0: kt * 2 * P2 + t0 + tsz],
                                wp[:, kt * 768 + n0: kt * 768 + n0 + nsz],
                                start=(kt == 0),
                                stop=(kt == KT - 1),
                            )
                        nc.vector.tensor_tensor(
                            osb[0:tsz, n0: n0 + nsz],
                            pp[0:tsz, 0:nsz],
                            bp[0:tsz, n0: n0 + nsz],
                            ALU.add,
                        )
                    nc.sync.dma_start(
                        out_d[pr * 2 * P2 + t0: pr * 2 * P2 + t0 + tsz, :],
                        osb[0:tsz, :],
                    )

            # ---- software-pipelined pair loop ----
            vproj(0)
            vproj(1)
            sdn2(0)
            hop2(0)
            vproj(2)
            vproj(3)
            mix(0)
            hop3(0)
            for pr in range(NPR):
                if pr + 1 < NPR:
                    sdn2(pr + 1)        # PE cover for hop3(pr) flight
                    hop2(pr + 1)
                av(2 * pr)
                av(2 * pr + 1)
                proj2(pr)
                if pr + 2 < NPR:
                    vproj(2 * pr + 4)   # PE cover for hop2(pr+1) flight
                    vproj(2 * pr + 5)
                if pr + 1 < NPR:
                    mix(pr + 1)
                    hop3(pr + 1)

    nc.compile()
    return nc


def _tile6(a, width):
    """[768, M] -> [128, 6*M] (K-tile-major host layout)."""
    assert a.shape == (768, width)
    return np.ascontiguousarray(
        a.reshape(KT, 128, width).transpose(1, 0, 2).reshape(128, KT * width)
    )


def _to_bf16(a):
    return np.asarray(a, dtype=np.float32).astype(ml_dtypes.bfloat16)


def _posmaps():
    """token m -> padded position p, and p -> m (or -1 for dummies)."""
    pos_of_tok = np.empty(N, np.int64)
    for m in range(N):
        c = 0 if m < 100 else 1
        mm = m - c * 100
        g, ml = mm // 10, mm % 10
        pos_of_tok[m] = c * 100 + ml * 10 + g
    tok_of_pos = np.full(P2, -1, np.int64)
    tok_of_pos[pos_of_tok] = np.arange(N)
    return pos_of_tok, tok_of_pos


_POS_OF_TOK, _TOK_OF_POS = _posmaps()


def _preprocess(inputs):
    x = np.asarray(inputs["x"], np.float32)
    qkv_w = np.asarray(inputs["qkv_w"], np.float32)
    q_bias = np.asarray(inputs["q_bias"], np.float32)
    v_bias = np.asarray(inputs["v_bias"], np.float32)
    sq = np.asarray(inputs["ssf_scale_qkv"], np.float32)
    tq = np.asarray(inputs["ssf_shift_qkv"], np.float32)
    rbt = np.asarray(inputs["rel_bias_table"], np.float32)
    coeff = np.asarray(inputs["bases_coeff"], np.float32)
    proj_w = np.asarray(inputs["proj_w"], np.float32)
    proj_b = np.asarray(inputs["proj_b"], np.float32)
    sp = np.asarray(inputs["ssf_scale_proj"], np.float32)
    tp = np.asarray(inputs["ssf_shift_proj"], np.float32)
    rel_index = np.asarray(inputs["rel_index"], np.int64)

    qkv_bias = np.concatenate([q_bias, np.zeros_like(q_bias), v_bias])
    w_eff = (qkv_w * sq[:, None]).copy()
    b_eff = (qkv_bias * sq + tq).copy()
    w_eff[0:768] *= SCALE
    b_eff[0:768] *= SCALE

    # wqk mt-major: [128, mt*768 + kt*128 + c]
    a = np.ascontiguousarray(w_eff[0:1536].T)          # [768, 1536]
    wqk = np.ascontiguousarray(
        a.reshape(KT, 128, QKM, 128).transpose(1, 2, 0, 3).reshape(128, QKM * 768)
    )
    wvt = _tile6(np.ascontiguousarray(w_eff[1536:].T), 768)
    wp_eff = proj_w * sp[:, None]
    bp_eff = proj_b * sp + tp
    # fold the v-bias through AV+proj: sum_l attn'[k,n,l] = m_k (constant)
    mix = coeff.T + np.eye(H, dtype=np.float32)
    m_k = mix.sum(axis=0)                              # [H]
    bv_eff = b_eff[1536:]
    bp_eff = bp_eff + wp_eff @ (np.repeat(m_k, DH) * bv_eff)
    wpt = _tile6(np.ascontiguousarray(wp_eff.T), 768)

    bqk_sb = np.ascontiguousarray(b_eff[0:1536].reshape(QKM, 128).T).astype(np.float32)

    # rel bias in permuted+padded coordinates:
    # relb[p, h*2*P2 + c*P2 + n] = exp(table[rel_index[qtok(n), ktok(c,p)], h])
    gathered = rbt[rel_index]                      # [query-tok, key-tok, H]
    relb4 = np.zeros((100, H, 2, P2), np.float32)
    q_valid = _TOK_OF_POS >= 0                     # [P2]
    qtok = np.where(q_valid, _TOK_OF_POS, 0)
    for c in range(2):
        ktok_pos = _TOK_OF_POS[c * 100: (c + 1) * 100]   # [100]
        k_valid = ktok_pos >= 0
        ktok = np.where(k_valid, ktok_pos, 0)
        blk = gathered[qtok[None, :], ktok[:, None], :]   # [100, P2, H]
        blk = blk.transpose(0, 2, 1)                      # [100, H, P2]
        blk = np.where(q_valid[None, None, :], blk, 0.0)
        blk = np.where(k_valid[:, None, None], blk, DUMMY_BIAS)
        relb4[:, :, c, :] = blk
    relb = np.exp(relb4.reshape(100, H * 2 * P2))

    # mixblk[wgi*12+h, wgi'*12+k] = d(wgi,wgi') mix[h,k]
    mixblk = np.kron(np.eye(10, dtype=np.float32), mix)
    bp_rep = np.broadcast_to(bp_eff.reshape(1, 768), (128, 768))

    common = {
        "wqk": _to_bf16(wqk),
        "wv": _to_bf16(wvt),
        "wp": _to_bf16(wpt),
        "relb": _to_bf16(relb),
        "mixblk": _to_bf16(mixblk),
        "bqk": bqk_sb,
        "bp": _to_bf16(bp_rep),
    }
    in_maps = []
    for ci in range(NCORES):
        xs = x[ci * BL: (ci + 1) * BL]              # [BL, N, C]
        xp = np.zeros((BL, P2, C), np.float32)
        xp[:, _POS_OF_TOK, :] = xs
        xt = xp.reshape(BL * P2, C).T               # [C, T2]
        # chunk-major xT: per chunk [128, 6*nsz], col = base6 + kt*nsz + n
        parts = []
        for (n0, nsz) in TOKC:
            blk = np.ascontiguousarray(xt[:, n0:n0 + nsz])
            parts.append(blk.reshape(KT, 128, nsz).transpose(1, 0, 2).reshape(128, KT * nsz))
        m = dict(common)
        m["xT"] = _to_bf16(np.concatenate(parts, axis=1))
        in_maps.append(m)
    return in_maps


def _get_compiled():
    if "nc" not in _COMPILED:
        _COMPILED["nc"] = _build_graph()
    return _COMPILED["nc"]


LAST_EXEC_NS = None
LAST_RESULTS = None


def _ensure_ntff_hook():
    """The agent image's antenv package lacks axon_hooks; synthesize it so
    run_bass_kernel_spmd(trace=True) can capture NTFF profiles."""
    import types

    if "antenv.axon_hooks" in sys.modules:
        return
    try:
        sys.path.insert(0, "/root/.axon_site")
        from trn_agent_boot.trn_boot import _ntff_profile_via_ctypes

        hook = _ntff_profile_via_ctypes("/opt/axon/libaxon_pjrt.so")
    except Exception:
        hook = None
    mod = types.ModuleType("antenv.axon_hooks")
    _state = {"hook": hook}
    mod.get_axon_ntff_profile_hook = lambda: _state["hook"]
    mod.set_axon_ntff_profile_hook = lambda h: _state.__setitem__("hook", h)
    sys.modules["antenv.axon_hooks"] = mod


def kernel(**inputs) -> np.ndarray:
    global LAST_EXEC_NS, LAST_RESULTS
    nc = _get_compiled()
    in_maps = _preprocess(inputs)
    from concourse.bass_utils import run_bass_kernel_spmd

    trace = os.environ.get("BASS_KERNEL_PROFILE", "0") == "1"
    if trace:
        _ensure_ntff_hook()
    res = run_bass_kernel_spmd(nc, in_maps, core_ids=list(range(NCORES)), trace=trace)
    LAST_EXEC_NS = res.exec_time_ns
    LAST_RESULTS = res
    outs = []
    for i in range(NCORES):
        o = np.asarray(res.results[i]["out"], dtype=np.float32).reshape(BL, P2, C)
        outs.append(o[:, _POS_OF_TOK, :])           # drop dummies, un-permute
    return np.concatenate(outs, axis=0).astype(np.float32)


# revision 40
# speedup vs baseline: 1.0893x; 1.0893x over previous
"""Trainium2 Bass kernel for nn_Attention_39608188404100 (v4.1).

Windowed-attention block (ViT-style, N=197 tokens) with SSF affines, relative
position bias, DCF head mixing, and output projection.

Strategy: pure data-parallel over batch across 8 NeuronCores (B=64 -> 8/core).
All weights replicated; no collectives. bf16 on the PE, fp32 PSUM.

Per core (BL=8 batches): each batch's 197 tokens are padded to 200 positions
and PERMUTED on host: position p = c*100 + ml*10 + g holds token
m = c*100 + g*10 + ml (c = chunk, 2x100). The 3 dummy positions get zero
x-columns and exp(-40) relative-bias multipliers, so they vanish in softmax;
dummy query columns are dropped on host after download.

v4.1 structure (no DRAM scratch; batch-PAIR pipeline to amortize the ~0.8us
per-DMA-trigger cost on the sync queue):
  - Stage 1: all 12 q/k channel-tile GEMMs back to back; uploads split so the
    first matmul starts after ~1MB.
  - Pair loop (pr = 0..3, batches 2pr/2pr+1): per pair, scores/exp/relb-mult/
    den(ones-matmul)/normalize build et2 [100, (h b2 cn)] (double-buffered);
    one-hop SBUF->SBUF shuffle (10 j-split DMAs, 1600B runs) to
    mxin2 [(wgi h), (j b2 cn)]; block-diag mix matmul; second one-hop shuffle
    (10 j-split DMAs) back to key-partition a2 (ALIASES et2); AV; 400-token
    projection (4 chunks) per pair.
  - All shuffle DMA access patterns are partition-contiguous on BOTH sides
    (the [120 x run] side is expressed as a plain 2D slice).
  - Drain work is spread over three engines: exp + AV drains on Scalar,
    norm/recip/mix drains on Vector, relb-mult + v/proj drains on GpSimd.
  - The v-bias is folded into the proj bias host-side (rows of the
    normalized+mixed attention sum to m_k = 1 + sum_h coeff[k,h]).

Env:
  BASS_KERNEL_PROFILE=1  capture neuron-profile (exec_time_ns) on the run.
"""
import os
import sys

sys.path.insert(0, "/opt/trn_rl_repo")

import numpy as np
import ml_dtypes

import concourse.bass as bass
import concourse.tile as tile
from concourse import bacc, mybir

BF16 = mybir.dt.bfloat16
F32 = mybir.dt.float32
AF = mybir.ActivationFunctionType
ALU = mybir.AluOpType

B, N, C, H, DH = 64, 197, 768, 12, 64
NCORES = 8
BL = B // NCORES          # 8 batches per core
NPR = BL // 2             # 4 batch pairs per core
P2 = 200                  # padded positions per batch
T2 = BL * P2              # 1600 positions per core
SCALE = DH ** -0.5
KT = 6                    # contraction tiles of 128 over C=768
QKM = 12                  # 128-wide M tiles over 1536 q/k channels
TOKC = [(0, 500), (500, 500), (1000, 500), (1500, 100)]  # 100-aligned chunks
TOKBASE = [0, 3000, 6000, 9000]
DUMMY_BIAS = -40.0
E2 = 2 * 2 * P2           # 800: per-head (b2, c, n) block in et2/a2
ECOLS2 = H * E2           # 9600: et2 cols
MCOLS2 = 10 * E2          # 8000: mxin2/mxo2 cols

_COMPILED = {}


def _build_graph():
    # detect_race_conditions=False: the sim race-detector's shadow model
    # linearizes multi-dim DMA APs as byte offsets and reports false overlaps
    # between distinct tiles; value semantics were validated in CoreSim and
    # against hardware.
    nc = bacc.Bacc(
        "TRN2", target_bir_lowering=False, debug=False,
        detect_race_conditions=False,
    )

    xT_d = nc.dram_tensor("xT", [128, KT * T2], BF16, kind="ExternalInput")
    wqk_d = nc.dram_tensor("wqk", [128, QKM * 768], BF16, kind="ExternalInput")
    wv_d = nc.dram_tensor("wv", [128, KT * 768], BF16, kind="ExternalInput")
    wp_d = nc.dram_tensor("wp", [128, KT * 768], BF16, kind="ExternalInput")
    relb_d = nc.dram_tensor("relb", [100, H * 2 * P2], BF16, kind="ExternalInput")
    mix_d = nc.dram_tensor("mixblk", [120, 120], BF16, kind="ExternalInput")
    bqk_d = nc.dram_tensor("bqk", [128, QKM], F32, kind="ExternalInput")
    bp_d = nc.dram_tensor("bp", [128, 768], BF16, kind="ExternalInput")
    out_d = nc.dram_tensor("out", [T2, 768], BF16, kind="ExternalOutput")

    with tile.TileContext(nc) as tc:
        with (
            tc.tile_pool(name="const", bufs=1) as cpool,
            tc.tile_pool(name="dch", bufs=2) as dpool,
            tc.tile_pool(name="osb", bufs=2) as opool,
            tc.tile_pool(name="psB", bufs=4, space=bass.MemorySpace.PSUM) as psB,
        ):
            # ---- persistent tiles ----
            xT = cpool.tile([128, KT * T2], BF16)
            qk_t = [cpool.tile([128, T2], BF16, name=f"qk{mt}")
                    for mt in range(QKM)]
            relb = cpool.tile([100, H * 2 * P2], BF16)
            wv = cpool.tile([128, KT * 768], BF16)
            wp = cpool.tile([128, KT * 768], BF16)
            mixblk = cpool.tile([120, 120], BF16)
            bqk = cpool.tile([128, QKM], F32)
            bp = cpool.tile([128, 768], BF16)
            ones_den = cpool.tile([128, 128], BF16)
            et2 = [cpool.tile([128, ECOLS2], BF16, name=f"et2_{s}")
                   for s in range(2)]                 # double-buffered; a2 alias
            mxin2 = cpool.tile([120, MCOLS2], BF16)
            mxo2 = cpool.tile([120, MCOLS2], BF16)
            aoT2 = [cpool.tile([128, KT * 2 * P2], BF16, name=f"aoT2_{s}")
                    for s in range(2)]                # out^T per pair
            vring = [cpool.tile([100, 2 * 768], BF16, name=f"v{s}")
                     for s in range(4)]

            nc.vector.memset(ones_den[:], 1.0)
            nc.vector.memset(et2[0][:], 0.0)
            nc.vector.memset(et2[1][:], 0.0)
            nc.vector.memset(mxin2[:], 0.0)
            nc.vector.memset(mxo2[:], 0.0)

            def xt_slice(t0, kt, w):
                """xT AP for tokens [t0, t0+w) at contraction tile kt."""
                ci = min(t0 // 500, 3)
                base, off, nsz_c = TOKBASE[ci], t0 - TOKC[ci][0], TOKC[ci][1]
                col = base + kt * nsz_c + off
                return xT[:, col: col + w]

            # ---- stage 1: q/k GEMMs, uploads overlapped ----
            with tc.tile_pool(name="wqk", bufs=1) as wpool:
                wqk = wpool.tile([128, QKM * 768], BF16)
                # critical-path uploads first
                nc.sync.dma_start(wqk[:, 0:768], wqk_d[:, 0:768])
                nc.sync.dma_start(xT[:, 0:3000], xT_d[:, 0:3000])
                nc.sync.dma_start(bqk[:], bqk_d[:])
                nc.sync.dma_start(wqk[:, 768:], wqk_d[:, 768:])
                nc.sync.dma_start(xT[:, 3000:], xT_d[:, 3000:])
                nc.sync.dma_start(relb[:], relb_d[:])
                nc.sync.dma_start(wv[:], wv_d[:])
                nc.sync.dma_start(mixblk[:], mix_d[:])
                nc.sync.dma_start(wp[:], wp_d[:])
                nc.sync.dma_start(bp[:], bp_d[:])

                for mt in range(QKM):
                    for ci, (n0, nsz) in enumerate(TOKC):
                        ps = psB.tile([128, 1024], F32, tag="b")
                        for kt in range(KT):
                            # nsz <= 500: output stays within one PSUM bank
                            nc.tensor.matmul(
                                ps[:, 0:nsz],
                                wqk[:, mt * 768 + kt * 128: mt * 768 + (kt + 1) * 128],
                                xT[:, TOKBASE[ci] + kt * nsz: TOKBASE[ci] + kt * nsz + nsz],
                                start=(kt == 0),
                                stop=(kt == KT - 1),
                            )
                        nc.scalar.activation(
                            qk_t[mt][:, n0:n0 + nsz],
                            ps[:, 0:nsz],
                            AF.Identity,
                            bias=bqk[:, mt:mt + 1],
                            scale=1.0,
                        )

            # ---- pair-loop pieces ----
            def vproj(b):
                """v for batch b -> vring[b % 4] (no bias: folded into bp)."""
                vt = vring[b % 4]
                for c in range(2):
                    t0 = b * P2 + c * 100
                    ps = psB.tile([128, 1024], F32, tag="b")
                    # two bank-contained accumulation groups, one drain
                    for n0, nsz in ((0, 512), (512, 256)):
                        for kt in range(KT):
                            nc.tensor.matmul(
                                ps[0:100, n0:n0 + nsz],
                                xt_slice(t0, kt, 100),
                                wv[:, kt * 768 + n0: kt * 768 + n0 + nsz],
                                start=(kt == 0),
                                stop=(kt == KT - 1),
                            )
                    nc.scalar.copy(
                        vt[0:100, c * 768: (c + 1) * 768],
                        ps[0:100, 0:768],
                    )

            def sdn(b):
                """scores -> exp -> *relb -> den -> normalize for batch b
                into the (h, b2, c, n) columns of et2[(b//2) % 2]."""
                b2 = b % 2
                et = et2[(b // 2) % 2][0:100, :]
                # [p, h, b2, c, n] and [p, h, b2, (c n)] views
                etv = et.rearrange("p (h b2 c n) -> p h b2 c n",
                                   h=H, b2=2, c=2, n=P2)
                etb = et.rearrange("p (h b2 cn) -> p h b2 cn",
                                   h=H, b2=2, cn=2 * P2)
                relv = relb[:].rearrange("p (h cn) -> p h cn", h=H, cn=2 * P2)

                def sc(tq):
                    for hp in range(2):
                        h0 = 4 * tq + 2 * hp
                        ps1 = psB.tile([128, 1024], F32, tag="b")
                        for hh in range(2):
                            # head hh at col hh*512 so each [100,200] matmul
                            # output stays within one PSUM bank
                            h = h0 + hh
                            prow = (h % 2) * 64
                            qt = qk_t[h // 2]
                            kt_ = qk_t[6 + h // 2]
                            for c in range(2):
                                nc.tensor.matmul(
                                    ps1[0:100, hh * 512 + c * P2:
                                        hh * 512 + (c + 1) * P2],
                                    kt_[prow:prow + 64,
                                        b * P2 + c * 100: b * P2 + c * 100 + 100],
                                    qt[prow:prow + 64, b * P2: b * P2 + P2],
                                    start=True, stop=True,
                                )
                        nc.scalar.activation(
                            etb[:, h0:h0 + 2, b2, :],
                            ps1[0:100, :].rearrange(
                                "p (h x) -> p h x", h=2)[:, :, 0:2 * P2],
                            AF.Exp,
                        )
                    nc.vector.tensor_tensor(
                        etb[:, 4 * tq:4 * tq + 4, b2, :],
                        etb[:, 4 * tq:4 * tq + 4, b2, :],
                        relv[:, 4 * tq:4 * tq + 4, :],
                        ALU.mult,
                    )

                def den(tq):
                    psd = psB.tile([128, 1024], F32, tag="b")
                    for dd in range(2):
                        # dd group at col dd*512 (bank-contained [100,400])
                        nc.tensor.matmul(
                            psd[0:100, dd * 512:dd * 512 + 400],
                            ones_den[0:100, 0:100],
                            etv[:, 4 * tq + 2 * dd: 4 * tq + 2 * dd + 2, b2, 0, :],
                            start=True, stop=False,
                        )
                        nc.tensor.matmul(
                            psd[0:100, dd * 512:dd * 512 + 400],
                            ones_den[0:100, 0:100],
                            etv[:, 4 * tq + 2 * dd: 4 * tq + 2 * dd + 2, b2, 1, :],
                            start=False, stop=True,
                        )
                    dch = dpool.tile([100, 800], F32, tag="dch")
                    for dd in range(2):
                        nc.vector.reciprocal_approx_fast(
                            dch[:, dd * 400:(dd + 1) * 400],
                            psd[0:100, dd * 512: dd * 512 + 400],
                        )
                    dv4 = dch[:].rearrange("p (h n) -> p h n", h=4)
                    for c in range(2):
                        nc.vector.tensor_tensor(
                            etv[:, 4 * tq:4 * tq + 4, b2, c, :],
                            etv[:, 4 * tq:4 * tq + 4, b2, c, :],
                            dv4, ALU.mult,
                        )

                sc(0)
                sc(1)
                den(0)
                sc(2)
                den(1)
                den(2)

            def sdn2(pr):
                sdn(2 * pr)
                sdn(2 * pr + 1)

            def hop2(pr):
                """et2 [(j wgi), (h b2 cn)] -> mxin2 [(wgi h), (j b2 cn)].
                j-split; both sides partition-contiguous; 1600B runs."""
                et_v = et2[pr % 2][0:100, :].rearrange(
                    "(j wgi) f -> j wgi f", j=10)
                for j in range(10):
                    nc.sync.dma_start(
                        mxin2[:, j * E2: (j + 1) * E2], et_v[j])

            def mix(pr):
                for i, o in enumerate(range(0, MCOLS2, 1000)):
                    psm = psB.tile([128, 1024], F32, tag="b")
                    for q in range(2):
                        nc.tensor.matmul(
                            psm[0:120, q * 512: q * 512 + 500], mixblk[:],
                            mxin2[:, o + q * 500: o + (q + 1) * 500],
                            start=True, stop=True,
                        )
                    pv2 = psm[0:120, :].rearrange(
                        "p (q x) -> p q x", q=2)[:, :, 0:500]
                    ov2 = mxo2[:, o:o + 1000].rearrange(
                        "p (q x) -> p q x", q=2)
                    if i % 4 != 0:
                        nc.scalar.copy(ov2, pv2)
                    else:
                        nc.vector.tensor_scalar_add(ov2, pv2, 0.0)

            def hop3(pr):
                """mxo2 [(wgi k), (j b2 cn)] -> a2 [(j wgi), (k b2 cn)].
                j-split; src is a plain [120, E2] slice; 1600B runs."""
                a2_v = et2[pr % 2][0:100, :].rearrange(
                    "(j wgi) (k f) -> j wgi k f", wgi=10, f=E2)
                for j in range(10):
                    nc.sync.dma_start(
                        a2_v[j], mxo2[:, j * E2: (j + 1) * E2])

            def av(b):
                b2 = b % 2
                a2 = et2[(b // 2) % 2][0:100, :]
                ao = aoT2[(b // 2) % 2]
                vt = vring[b % 4]
                # jj groups of (4, 2) head-pairs share one PSUM tile each;
                # within a group, jj at col (jj%2)*P2 + (jj//2 within grp)*512
                for g0, gn in ((0, 4), (4, 2)):
                    pv = psB.tile([128, 1024], F32, tag="b")
                    for gi in range(gn):
                        jj = g0 + gi
                        col = (gi // 2) * 512 + (gi % 2) * P2
                        for sub in range(2):
                            k = 2 * jj + sub
                            rows = pv[sub * 64: sub * 64 + 64, col: col + P2]
                            tp = (0, sub * 64)
                            for c in range(2):
                                nc.tensor.matmul(
                                    rows,
                                    vt[0:100, c * 768 + k * 64: c * 768 + (k + 1) * 64],
                                    a2[0:100, k * E2 + b2 * 2 * P2 + c * P2:
                                       k * E2 + b2 * 2 * P2 + c * P2 + P2],
                                    start=(c == 0),
                                    stop=(c == 1),
                                    tile_position=tp,
                                )
                    # drain per bank-pair: [128, 2, P2] strided copy
                    aov = ao[:].rearrange("p (jj t) -> p jj t", t=2 * P2)
                    for q in range((gn + 1) // 2):
                        nc.scalar.copy(
                            aov[:, g0 + 2 * q: g0 + 2 * q + 2,
                                b2 * P2:(b2 + 1) * P2],
                            pv[:, q * 512: q * 512 + 2 * P2].rearrange(
                                "p (jj n) -> p jj n", n=P2),
                        )

            def proj2(pr):
                """output projection for the pair's 400 tokens, 4 chunks."""
                ao = aoT2[pr % 2]
                for t0, tsz in ((0, 128), (128, 128), (256, 128), (384, 16)):
                    osb = opool.tile([128, 768], BF16, tag="osb")
                    pp = psB.tile([128, 1024], F32, tag="b")
                    for n0, nsz in ((0, 512), (512, 256)):
                        for kt in range(KT):
                            nc.tensor.matmul(
                                pp[0:tsz, n0:n0 + nsz],
                                ao[:, kt * 2 * P2 + t0: kt * 2 * P2 + t0 + tsz],
                                wp[:, kt * 768 + n0: kt * 768 + n0 + nsz],
                                start=(kt == 0),
                                stop=(kt == KT - 1),
                            )
                    nc.vector.tensor_tensor(
                        osb[0:tsz, 0:768],
                        pp[0:tsz, 0:768],
                        bp[0:tsz, 0:768],
                        ALU.add,
                    )
                    nc.sync.dma_start(
                        out_d[pr * 2 * P2 + t0: pr * 2 * P2 + t0 + tsz, :],
                        osb[0:tsz, :],
                    )

            # ---- software-pipelined pair loop ----
            vproj(0)
            vproj(1)
            sdn2(0)
            hop2(0)
            vproj(2)
            vproj(3)
            mix(0)
            hop3(0)
            for pr in range(NPR):
                if pr + 1 < NPR:
                    sdn2(pr + 1)        # PE cover for hop3(pr) flight
                    hop2(pr + 1)
                av(2 * pr)
                av(2 * pr + 1)
                proj2(pr)
                if pr + 2 < NPR:
                    vproj(2 * pr + 4)   # PE cover for hop2(pr+1) flight
                    vproj(2 * pr + 5)
                if pr + 1 < NPR:
                    mix(pr + 1)
                    hop3(pr + 1)

    nc.compile()
    return nc


def _tile6(a, width):
    """[768, M] -> [128, 6*M] (K-tile-major host layout)."""
    assert a.shape == (768, width)
    return np.ascontiguousarray(
        a.reshape(KT, 128, width).transpose(1, 0, 2).reshape(128, KT * width)
    )


def _to_bf16(a):
    return np.asarray(a, dtype=np.float32).astype(ml_dtypes.bfloat16)


def _posmaps():
    """token m -> padded position p, and p -> m (or -1 for dummies)."""
    pos_of_tok = np.empty(N, np.int64)
    for m in range(N):
        c = 0 if m < 100 else 1
        mm = m - c * 100
        g, ml = mm // 10, mm % 10
        pos_of_tok[m] = c * 100 + ml * 10 + g
    tok_of_pos = np.full(P2, -1, np.int64)
    tok_of_pos[pos_of_tok] = np.arange(N)
    return pos_of_tok, tok_of_pos


_POS_OF_TOK, _TOK_OF_POS = _posmaps()


def _preprocess(inputs):
    x = np.asarray(inputs["x"], np.float32)
    qkv_w = np.asarray(inputs["qkv_w"], np.float32)
    q_bias = np.asarray(inputs["q_bias"], np.float32)
    v_bias = np.asarray(inputs["v_bias"], np.float32)
    sq = np.asarray(inputs["ssf_scale_qkv"], np.float32)
    tq = np.asarray(inputs["ssf_shift_qkv"], np.float32)
    rbt = np.asarray(inputs["rel_bias_table"], np.float32)
    coeff = np.asarray(inputs["bases_coeff"], np.float32)
    proj_w = np.asarray(inputs["proj_w"], np.float32)
    proj_b = np.asarray(inputs["proj_b"], np.float32)
    sp = np.asarray(inputs["ssf_scale_proj"], np.float32)
    tp = np.asarray(inputs["ssf_shift_proj"], np.float32)
    rel_index = np.asarray(inputs["rel_index"], np.int64)

    qkv_bias = np.concatenate([q_bias, np.zeros_like(q_bias), v_bias])
    w_eff = (qkv_w * sq[:, None]).copy()
    b_eff = (qkv_bias * sq + tq).copy()
    w_eff[0:768] *= SCALE
    b_eff[0:768] *= SCALE

    # wqk mt-major: [128, mt*768 + kt*128 + c]
    a = np.ascontiguousarray(w_eff[0:1536].T)          # [768, 1536]
    wqk = np.ascontiguousarray(
        a.reshape(KT, 128, QKM, 128).transpose(1, 2, 0, 3).reshape(128, QKM * 768)
    )
    wvt = _tile6(np.ascontiguousarray(w_eff[1536:].T), 768)
    wp_eff = proj_w * sp[:, None]
    bp_eff = proj_b * sp + tp
    # fold the v-bias through AV+proj: sum_l attn'[k,n,l] = m_k (constant)
    mix = coeff.T + np.eye(H, dtype=np.float32)
    m_k = mix.sum(axis=0)                              # [H]
    bv_eff = b_eff[1536:]
    bp_eff = bp_eff + wp_eff @ (np.repeat(m_k, DH) * bv_eff)
    wpt = _tile6(np.ascontiguousarray(wp_eff.T), 768)

    bqk_sb = np.ascontiguousarray(b_eff[0:1536].reshape(QKM, 128).T).astype(np.float32)

    # rel bias in permuted+padded coordinates:
    # relb[p, h*2*P2 + c*P2 + n] = exp(table[rel_index[qtok(n), ktok(c,p)], h])
    gathered = rbt[rel_index]                      # [query-tok, key-tok, H]
    relb4 = np.zeros((100, H, 2, P2), np.float32)
    q_valid = _TOK_OF_POS >= 0                     # [P2]
    qtok = np.where(q_valid, _TOK_OF_POS, 0)
    for c in range(2):
        ktok_pos = _TOK_OF_POS[c * 100: (c + 1) * 100]   # [100]
        k_valid = ktok_pos >= 0
        ktok = np.where(k_valid, ktok_pos, 0)
        blk = gathered[qtok[None, :], ktok[:, None], :]   # [100, P2, H]
        blk = blk.transpose(0, 2, 1)                      # [100, H, P2]
        blk = np.where(q_valid[None, None, :], blk, 0.0)
        blk = np.where(k_valid[:, None, None], blk, DUMMY_BIAS)
        relb4[:, :, c, :] = blk
    relb = np.exp(relb4.reshape(100, H * 2 * P2))

    # mixblk[wgi*12+h, wgi'*12+k] = d(wgi,wgi') mix[h,k]
    mixblk = np.kron(np.eye(10, dtype=np.float32), mix)
    bp_rep = np.broadcast_to(bp_eff.reshape(1, 768), (128, 768))

    common = {
        "wqk": _to_bf16(wqk),
        "wv": _to_bf16(wvt),
        "wp": _to_bf16(wpt),
        "relb": _to_bf16(relb),
        "mixblk": _to_bf16(mixblk),
        "bqk": bqk_sb,
        "bp": _to_bf16(bp_rep),
    }
    in_maps = []
    for ci in range(NCORES):
        xs = x[ci * BL: (ci + 1) * BL]              # [BL, N, C]
        xp = np.zeros((BL, P2, C), np.float32)
        xp[:, _POS_OF_TOK, :] = xs
        xt = xp.reshape(BL * P2, C).T               # [C, T2]
        # chunk-major xT: per chunk [128, 6*nsz], col = base6 + kt*nsz + n
        parts = []
        for (n0, nsz) in TOKC:
            blk = np.ascontiguousarray(xt[:, n0:n0 + nsz])
            parts.append(blk.reshape(KT, 128, nsz).transpose(1, 0, 2).reshape(128, KT * nsz))
        m = dict(common)
        m["xT"] = _to_bf16(np.concatenate(parts, axis=1))
        in_maps.append(m)
    return in_maps


def _get_compiled():
    if "nc" not in _COMPILED:
        _COMPILED["nc"] = _build_graph()
    return _COMPILED["nc"]


LAST_EXEC_NS = None
LAST_RESULTS = None


def _ensure_ntff_hook():
    """The agent image's antenv package lacks axon_hooks; synthesize it so
    run_bass_kernel_spmd(trace=True) can capture NTFF profiles."""
    import types

    if "antenv.axon_hooks" in sys.modules:
        return
    try:
        sys.path.insert(0, "/root/.axon_site")
        from trn_agent_boot.trn_boot import _ntff_profile_via_ctypes

        hook = _ntff_profile_via_ctypes("/opt/axon/libaxon_pjrt.so")
    except Exception:
        hook = None
    mod = types.ModuleType("antenv.axon_hooks")
    _state = {"hook": hook}
    mod.get_axon_ntff_profile_hook = lambda: _state["hook"]
    mod.set_axon_ntff_profile_hook = lambda h: _state.__setitem__("hook", h)
    sys.modules["antenv.axon_hooks"] = mod


def kernel(**inputs) -> np.ndarray:
    global LAST_EXEC_NS, LAST_RESULTS
    nc = _get_compiled()
    in_maps = _preprocess(inputs)
    from concourse.bass_utils import run_bass_kernel_spmd

    trace = os.environ.get("BASS_KERNEL_PROFILE", "0") == "1"
    if trace:
        _ensure_ntff_hook()
    res = run_bass_kernel_spmd(nc, in_maps, core_ids=list(range(NCORES)), trace=trace)
    LAST_EXEC_NS = res.exec_time_ns
    LAST_RESULTS = res
    outs = []
    for i in range(NCORES):
        o = np.asarray(res.results[i]["out"], dtype=np.float32).reshape(BL, P2, C)
        outs.append(o[:, _POS_OF_TOK, :])           # drop dummies, un-permute
    return np.concatenate(outs, axis=0).astype(np.float32)


# revision 44
# speedup vs baseline: 1.1755x; 1.0791x over previous
"""Trainium2 Bass kernel for nn_Attention_39608188404100 (v4.1).

Windowed-attention block (ViT-style, N=197 tokens) with SSF affines, relative
position bias, DCF head mixing, and output projection.

Strategy: pure data-parallel over batch across 8 NeuronCores (B=64 -> 8/core).
All weights replicated; no collectives. bf16 on the PE, fp32 PSUM.

Per core (BL=8 batches): each batch's 197 tokens are padded to 200 positions
and PERMUTED on host: position p = c*100 + ml*10 + g holds token
m = c*100 + g*10 + ml (c = chunk, 2x100). The 3 dummy positions get zero
x-columns and exp(-40) relative-bias multipliers, so they vanish in softmax;
dummy query columns are dropped on host after download.

v4.1 structure (no DRAM scratch; batch-PAIR pipeline to amortize the ~0.8us
per-DMA-trigger cost on the sync queue):
  - Stage 1: all 12 q/k channel-tile GEMMs back to back; uploads split so the
    first matmul starts after ~1MB.
  - Pair loop (pr = 0..3, batches 2pr/2pr+1): per pair, scores/exp/relb-mult/
    den(ones-matmul)/normalize build et2 [100, (h b2 cn)] (double-buffered);
    one-hop SBUF->SBUF shuffle (10 j-split DMAs, 1600B runs) to
    mxin2 [(wgi h), (j b2 cn)]; block-diag mix matmul; second one-hop shuffle
    (10 j-split DMAs) back to key-partition a2 (ALIASES et2); AV; 400-token
    projection (4 chunks) per pair.
  - All shuffle DMA access patterns are partition-contiguous on BOTH sides
    (the [120 x run] side is expressed as a plain 2D slice).
  - Drain work is spread over three engines: exp + AV drains on Scalar,
    norm/recip/mix drains on Vector, relb-mult + v/proj drains on GpSimd.
  - The v-bias is folded into the proj bias host-side (rows of the
    normalized+mixed attention sum to m_k = 1 + sum_h coeff[k,h]).

Env:
  BASS_KERNEL_PROFILE=1  capture neuron-profile (exec_time_ns) on the run.
"""
import os
import sys

sys.path.insert(0, "/opt/trn_rl_repo")

import numpy as np
import ml_dtypes

import concourse.bass as bass
import concourse.tile as tile
from concourse import bacc, mybir

BF16 = mybir.dt.bfloat16
F32 = mybir.dt.float32
AF = mybir.ActivationFunctionType
ALU = mybir.AluOpType

B, N, C, H, DH = 64, 197, 768, 12, 64
NCORES = 8
BL = B // NCORES          # 8 batches per core
NPR = BL // 2             # 4 batch pairs per core
P2 = 200                  # padded positions per batch
T2 = BL * P2              # 1600 positions per core
SCALE = DH ** -0.5
KT = 6                    # contraction tiles of 128 over C=768
QKM = 12                  # 128-wide M tiles over 1536 q/k channels
TOKC = [(0, 500), (500, 500), (1000, 500), (1500, 100)]  # 100-aligned chunks
TOKBASE = [0, 3000, 6000, 9000]
DUMMY_BIAS = -40.0
E2 = 2 * 2 * P2           # 800: per-head (b2, c, n) block in et2/a2
ECOLS2 = H * E2           # 9600: et2 cols
MCOLS2 = 10 * E2          # 8000: mxin2/mxo2 cols

_COMPILED = {}


def _build_graph():
    # detect_race_conditions=False: the sim race-detector's shadow model
    # linearizes multi-dim DMA APs as byte offsets and reports false overlaps
    # between distinct tiles; value semantics were validated in CoreSim and
    # against hardware.
    nc = bacc.Bacc(
        "TRN2", target_bir_lowering=False, debug=False,
        detect_race_conditions=False,
    )

    xT_d = nc.dram_tensor("xT", [128, KT * T2], BF16, kind="ExternalInput")
    wqk_d = nc.dram_tensor("wqk", [128, QKM * 768], BF16, kind="ExternalInput")
    wv_d = nc.dram_tensor("wv", [128, KT * 768], BF16, kind="ExternalInput")
    wp_d = nc.dram_tensor("wp", [128, KT * 768], BF16, kind="ExternalInput")
    relb_d = nc.dram_tensor("relb", [100, H * 2 * P2], BF16, kind="ExternalInput")
    mix_d = nc.dram_tensor("mixblk", [120, 120], BF16, kind="ExternalInput")
    bqk_d = nc.dram_tensor("bqk", [128, QKM], F32, kind="ExternalInput")
    bp_d = nc.dram_tensor("bp", [128, 768], BF16, kind="ExternalInput")
    out_d = nc.dram_tensor("out", [T2, 768], BF16, kind="ExternalOutput")

    with tile.TileContext(nc) as tc:
        with (
            tc.tile_pool(name="const", bufs=1) as cpool,
            tc.tile_pool(name="dch", bufs=2) as dpool,
            tc.tile_pool(name="osb", bufs=2) as opool,
            tc.tile_pool(name="psB", bufs=4, space=bass.MemorySpace.PSUM) as psB,
        ):
            # ---- persistent tiles ----
            xT = cpool.tile([128, KT * T2], BF16)
            qk_t = [cpool.tile([128, T2], BF16, name=f"qk{mt}")
                    for mt in range(QKM)]
            relb = cpool.tile([100, H * 2 * P2], BF16)
            wv = cpool.tile([128, KT * 768], BF16)
            wp = cpool.tile([128, KT * 768], BF16)
            mixblk = cpool.tile([120, 120], BF16)
            bqk = cpool.tile([128, QKM], F32)
            bp = cpool.tile([128, 768], BF16)
            ones_den = cpool.tile([128, 128], BF16)
            et2 = [cpool.tile([128, ECOLS2], BF16, name=f"et2_{s}")
                   for s in range(2)]                 # double-buffered; a2 alias
            mxin2 = cpool.tile([120, MCOLS2], BF16)
            mxo2 = cpool.tile([120, MCOLS2], BF16)
            aoT2 = [cpool.tile([128, KT * 2 * P2], BF16, name=f"aoT2_{s}")
                    for s in range(2)]                # out^T per pair
            vring = [cpool.tile([100, 2 * 768], BF16, name=f"v{s}")
                     for s in range(4)]

            nc.vector.memset(ones_den[:], 1.0)
            nc.vector.memset(et2[0][:], 0.0)
            nc.vector.memset(et2[1][:], 0.0)
            nc.vector.memset(mxin2[:], 0.0)
            nc.vector.memset(mxo2[:], 0.0)

            def xt_slice(t0, kt, w):
                """xT AP for tokens [t0, t0+w) at contraction tile kt."""
                ci = min(t0 // 500, 3)
                base, off, nsz_c = TOKBASE[ci], t0 - TOKC[ci][0], TOKC[ci][1]
                col = base + kt * nsz_c + off
                return xT[:, col: col + w]

            # ---- stage 1: q/k GEMMs, uploads overlapped ----
            with tc.tile_pool(name="wqk", bufs=1) as wpool:
                wqk = wpool.tile([128, QKM * 768], BF16)
                # critical-path uploads first
                nc.sync.dma_start(wqk[:, 0:768], wqk_d[:, 0:768])
                nc.sync.dma_start(xT[:, 0:3000], xT_d[:, 0:3000])
                nc.sync.dma_start(bqk[:], bqk_d[:])
                nc.sync.dma_start(wqk[:, 768:], wqk_d[:, 768:])
                nc.sync.dma_start(xT[:, 3000:], xT_d[:, 3000:])
                nc.sync.dma_start(relb[:], relb_d[:])
                nc.sync.dma_start(wv[:], wv_d[:])
                nc.sync.dma_start(mixblk[:], mix_d[:])
                nc.sync.dma_start(wp[:], wp_d[:])
                nc.sync.dma_start(bp[:], bp_d[:])

                for mt in range(QKM):
                    for ci, (n0, nsz) in enumerate(TOKC):
                        ps = psB.tile([128, 1024], F32, tag="b")
                        for kt in range(KT):
                            # nsz <= 500: output stays within one PSUM bank
                            nc.tensor.matmul(
                                ps[:, 0:nsz],
                                wqk[:, mt * 768 + kt * 128: mt * 768 + (kt + 1) * 128],
                                xT[:, TOKBASE[ci] + kt * nsz: TOKBASE[ci] + kt * nsz + nsz],
                                start=(kt == 0),
                                stop=(kt == KT - 1),
                            )
                        nc.scalar.activation(
                            qk_t[mt][:, n0:n0 + nsz],
                            ps[:, 0:nsz],
                            AF.Identity,
                            bias=bqk[:, mt:mt + 1],
                            scale=1.0,
                        )

            # ---- pair-loop pieces ----
            def vproj(b):
                """v for batch b -> vring[b % 4] (no bias: folded into bp)."""
                vt = vring[b % 4]
                for c in range(2):
                    t0 = b * P2 + c * 100
                    ps = psB.tile([128, 1024], F32, tag="b")
                    # two bank-contained accumulation groups, one drain
                    for n0, nsz in ((0, 512), (512, 256)):
                        for kt in range(KT):
                            nc.tensor.matmul(
                                ps[0:100, n0:n0 + nsz],
                                xt_slice(t0, kt, 100),
                                wv[:, kt * 768 + n0: kt * 768 + n0 + nsz],
                                start=(kt == 0),
                                stop=(kt == KT - 1),
                            )
                    nc.scalar.copy(
                        vt[0:100, c * 768: (c + 1) * 768],
                        ps[0:100, 0:768],
                    )

            def sdn(b):
                """scores -> exp -> *relb -> den -> normalize for batch b
                into the (h, b2, c, n) columns of et2[(b//2) % 2]."""
                b2 = b % 2
                et = et2[(b // 2) % 2][0:100, :]
                # [p, h, b2, c, n] and [p, h, b2, (c n)] views
                etv = et.rearrange("p (h b2 c n) -> p h b2 c n",
                                   h=H, b2=2, c=2, n=P2)
                etb = et.rearrange("p (h b2 cn) -> p h b2 cn",
                                   h=H, b2=2, cn=2 * P2)
                relv = relb[:].rearrange("p (h cn) -> p h cn", h=H, cn=2 * P2)

                def sc(tq):
                    for hp in range(2):
                        h0 = 4 * tq + 2 * hp
                        ps1 = psB.tile([128, 1024], F32, tag="b")
                        for hh in range(2):
                            # head hh at col hh*512 so each [100,200] matmul
                            # output stays within one PSUM bank
                            h = h0 + hh
                            prow = (h % 2) * 64
                            qt = qk_t[h // 2]
                            kt_ = qk_t[6 + h // 2]
                            for c in range(2):
                                nc.tensor.matmul(
                                    ps1[0:100, hh * 512 + c * P2:
                                        hh * 512 + (c + 1) * P2],
                                    kt_[prow:prow + 64,
                                        b * P2 + c * 100: b * P2 + c * 100 + 100],
                                    qt[prow:prow + 64, b * P2: b * P2 + P2],
                                    start=True, stop=True,
                                )
                        nc.scalar.activation(
                            etb[:, h0:h0 + 2, b2, :],
                            ps1[0:100, :].rearrange(
                                "p (h x) -> p h x", h=2)[:, :, 0:2 * P2],
                            AF.Exp,
                        )
                    nc.vector.tensor_tensor(
                        etb[:, 4 * tq:4 * tq + 4, b2, :],
                        etb[:, 4 * tq:4 * tq + 4, b2, :],
                        relv[:, 4 * tq:4 * tq + 4, :],
                        ALU.mult,
                    )

                def den(tq):
                    psd = psB.tile([128, 1024], F32, tag="b")
                    for dd in range(2):
                        # dd group at col dd*512 (bank-contained [100,400])
                        nc.tensor.matmul(
                            psd[0:100, dd * 512:dd * 512 + 400],
                            ones_den[0:100, 0:100],
                            etv[:, 4 * tq + 2 * dd: 4 * tq + 2 * dd + 2, b2, 0, :],
                            start=True, stop=False,
                        )
                        nc.tensor.matmul(
                            psd[0:100, dd * 512:dd * 512 + 400],
                            ones_den[0:100, 0:100],
                            etv[:, 4 * tq + 2 * dd: 4 * tq + 2 * dd + 2, b2, 1, :],
                            start=False, stop=True,
                        )
                    dch = dpool.tile([100, 800], F32, tag="dch")
                    for dd in range(2):
                        nc.vector.reciprocal_approx_fast(
                            dch[:, dd * 400:(dd + 1) * 400],
                            psd[0:100, dd * 512: dd * 512 + 400],
                        )
                    dv4 = dch[:].rearrange("p (h n) -> p h n", h=4)
                    for c in range(2):
                        nc.vector.tensor_tensor(
                            etv[:, 4 * tq:4 * tq + 4, b2, c, :],
                            etv[:, 4 * tq:4 * tq + 4, b2, c, :],
                            dv4, ALU.mult,
                        )

                sc(0)
                sc(1)
                den(0)
                sc(2)
                den(1)
                den(2)

            def sdn2(pr):
                sdn(2 * pr)
                sdn(2 * pr + 1)

            def hop2(pr):
                """et2 [(j wgi), (h b2 cn)] -> mxin2 [(wgi h), (j b2 cn)].
                j-split; both sides partition-contiguous; 1600B runs.
                Triggers split across the SP hwdge queue and the gpsimd
                SWDGE queue so the two 5-trigger bursts run in parallel."""
                et_v = et2[pr % 2][0:100, :].rearrange(
                    "(j wgi) f -> j wgi f", j=10)
                for j in range(10):
                    eng = nc.sync if j % 2 == 0 else nc.gpsimd
                    eng.dma_start(
                        mxin2[:, j * E2: (j + 1) * E2], et_v[j])

            def mix(pr):
                for i, o in enumerate(range(0, MCOLS2, 1000)):
                    psm = psB.tile([128, 1024], F32, tag="b")
                    for q in range(2):
                        nc.tensor.matmul(
                            psm[0:120, q * 512: q * 512 + 500], mixblk[:],
                            mxin2[:, o + q * 500: o + (q + 1) * 500],
                            start=True, stop=True,
                        )
                    pv2 = psm[0:120, :].rearrange(
                        "p (q x) -> p q x", q=2)[:, :, 0:500]
                    ov2 = mxo2[:, o:o + 1000].rearrange(
                        "p (q x) -> p q x", q=2)
                    if i % 4 != 0:
                        nc.scalar.copy(ov2, pv2)
                    else:
                        nc.vector.tensor_scalar_add(ov2, pv2, 0.0)

            def hop3(pr):
                """mxo2 [(wgi k), (j b2 cn)] -> a2 [(j wgi), (k b2 cn)].
                j-split; src is a plain [120, E2] slice; 1600B runs."""
                a2_v = et2[pr % 2][0:100, :].rearrange(
                    "(j wgi) (k f) -> j wgi k f", wgi=10, f=E2)
                for j in range(10):
                    eng = nc.sync if j % 2 == 0 else nc.gpsimd
                    eng.dma_start(
                        a2_v[j], mxo2[:, j * E2: (j + 1) * E2])

            def av(b):
                b2 = b % 2
                a2 = et2[(b // 2) % 2][0:100, :]
                ao = aoT2[(b // 2) % 2]
                vt = vring[b % 4]
                # jj groups of (4, 2) head-pairs share one PSUM tile each;
                # within a group, jj at col (jj%2)*P2 + (jj//2 within grp)*512
                for g0, gn in ((0, 4), (4, 2)):
                    pv = psB.tile([128, 1024], F32, tag="b")
                    for gi in range(gn):
                        jj = g0 + gi
                        col = (gi // 2) * 512 + (gi % 2) * P2
                        for sub in range(2):
                            k = 2 * jj + sub
                            rows = pv[sub * 64: sub * 64 + 64, col: col + P2]
                            tp = (0, sub * 64)
                            for c in range(2):
                                nc.tensor.matmul(
                                    rows,
                                    vt[0:100, c * 768 + k * 64: c * 768 + (k + 1) * 64],
                                    a2[0:100, k * E2 + b2 * 2 * P2 + c * P2:
                                       k * E2 + b2 * 2 * P2 + c * P2 + P2],
                                    start=(c == 0),
                                    stop=(c == 1),
                                    tile_position=tp,
                                )
                    # drain per bank-pair: [128, 2, P2] strided copy
                    aov = ao[:].rearrange("p (jj t) -> p jj t", t=2 * P2)
                    for q in range((gn + 1) // 2):
                        nc.scalar.copy(
                            aov[:, g0 + 2 * q: g0 + 2 * q + 2,
                                b2 * P2:(b2 + 1) * P2],
                            pv[:, q * 512: q * 512 + 2 * P2].rearrange(
                                "p (jj n) -> p jj n", n=P2),
                        )

            def proj2(pr, chunks=((0, 128), (128, 128), (256, 128), (384, 16))):
                """output projection for the pair's 400 tokens."""
                ao = aoT2[pr % 2]
                for t0, tsz in chunks:
                    osb = opool.tile([128, 768], BF16, tag="osb")
                    pp = psB.tile([128, 1024], F32, tag="b")
                    for n0, nsz in ((0, 512), (512, 256)):
                        for kt in range(KT):
                            nc.tensor.matmul(
                                pp[0:tsz, n0:n0 + nsz],
                                ao[:, kt * 2 * P2 + t0: kt * 2 * P2 + t0 + tsz],
                                wp[:, kt * 768 + n0: kt * 768 + n0 + nsz],
                                start=(kt == 0),
                                stop=(kt == KT - 1),
                            )
                    nc.vector.tensor_tensor(
                        osb[0:tsz, 0:768],
                        pp[0:tsz, 0:768],
                        bp[0:tsz, 0:768],
                        ALU.add,
                    )
                    nc.sync.dma_start(
                        out_d[pr * 2 * P2 + t0: pr * 2 * P2 + t0 + tsz, :],
                        osb[0:tsz, :],
                    )

            # ---- software-pipelined pair loop ----
            # per iter: sdn2(pr+1) covers hop3(pr) flight; av/proj/vproj
            # cover the hop2(pr+1) trigger burst; the trailing proj/vproj
            # chunks cover the hop3(pr+1) burst into the next iteration.
            vproj(0)
            vproj(1)
            sdn2(0)
            hop2(0)
            vproj(2)
            vproj(3)
            mix(0)
            hop3(0)
            CH = ((0, 128), (128, 128), (256, 128), (384, 16))
            for pr in range(NPR):
                if pr + 1 < NPR:
                    sdn2(pr + 1)
                    hop2(pr + 1)
                av(2 * pr)
                av(2 * pr + 1)
                proj2(pr, CH[0:2])
                if pr + 2 < NPR:
                    vproj(2 * pr + 4)
                if pr + 1 < NPR:
                    mix(pr + 1)
                    hop3(pr + 1)
                proj2(pr, CH[2:4])
                if pr + 2 < NPR:
                    vproj(2 * pr + 5)

    nc.compile()
    return nc


def _tile6(a, width):
    """[768, M] -> [128, 6*M] (K-tile-major host layout)."""
    assert a.shape == (768, width)
    return np.ascontiguousarray(
        a.reshape(KT, 128, width).transpose(1, 0, 2).reshape(128, KT * width)
    )


def _to_bf16(a):
    return np.asarray(a, dtype=np.float32).astype(ml_dtypes.bfloat16)


def _posmaps():
    """token m -> padded position p, and p -> m (or -1 for dummies)."""
    pos_of_tok = np.empty(N, np.int64)
    for m in range(N):
        c = 0 if m < 100 else 1
        mm = m - c * 100
        g, ml = mm // 10, mm % 10
        pos_of_tok[m] = c * 100 + ml * 10 + g
    tok_of_pos = np.full(P2, -1, np.int64)
    tok_of_pos[pos_of_tok] = np.arange(N)
    return pos_of_tok, tok_of_pos


_POS_OF_TOK, _TOK_OF_POS = _posmaps()


def _preprocess(inputs):
    x = np.asarray(inputs["x"], np.float32)
    qkv_w = np.asarray(inputs["qkv_w"], np.float32)
    q_bias = np.asarray(inputs["q_bias"], np.float32)
    v_bias = np.asarray(inputs["v_bias"], np.float32)
    sq = np.asarray(inputs["ssf_scale_qkv"], np.float32)
    tq = np.asarray(inputs["ssf_shift_qkv"], np.float32)
    rbt = np.asarray(inputs["rel_bias_table"], np.float32)
    coeff = np.asarray(inputs["bases_coeff"], np.float32)
    proj_w = np.asarray(inputs["proj_w"], np.float32)
    proj_b = np.asarray(inputs["proj_b"], np.float32)
    sp = np.asarray(inputs["ssf_scale_proj"], np.float32)
    tp = np.asarray(inputs["ssf_shift_proj"], np.float32)
    rel_index = np.asarray(inputs["rel_index"], np.int64)

    qkv_bias = np.concatenate([q_bias, np.zeros_like(q_bias), v_bias])
    w_eff = (qkv_w * sq[:, None]).copy()
    b_eff = (qkv_bias * sq + tq).copy()
    w_eff[0:768] *= SCALE
    b_eff[0:768] *= SCALE

    # wqk mt-major: [128, mt*768 + kt*128 + c]
    a = np.ascontiguousarray(w_eff[0:1536].T)          # [768, 1536]
    wqk = np.ascontiguousarray(
        a.reshape(KT, 128, QKM, 128).transpose(1, 2, 0, 3).reshape(128, QKM * 768)
    )
    wvt = _tile6(np.ascontiguousarray(w_eff[1536:].T), 768)
    wp_eff = proj_w * sp[:, None]
    bp_eff = proj_b * sp + tp
    # fold the v-bias through AV+proj: sum_l attn'[k,n,l] = m_k (constant)
    mix = coeff.T + np.eye(H, dtype=np.float32)
    m_k = mix.sum(axis=0)                              # [H]
    bv_eff = b_eff[1536:]
    bp_eff = bp_eff + wp_eff @ (np.repeat(m_k, DH) * bv_eff)
    wpt = _tile6(np.ascontiguousarray(wp_eff.T), 768)

    bqk_sb = np.ascontiguousarray(b_eff[0:1536].reshape(QKM, 128).T).astype(np.float32)

    # rel bias in permuted+padded coordinates:
    # relb[p, h*2*P2 + c*P2 + n] = exp(table[rel_index[qtok(n), ktok(c,p)], h])
    gathered = rbt[rel_index]                      # [query-tok, key-tok, H]
    relb4 = np.zeros((100, H, 2, P2), np.float32)
    q_valid = _TOK_OF_POS >= 0                     # [P2]
    qtok = np.where(q_valid, _TOK_OF_POS, 0)
    for c in range(2):
        ktok_pos = _TOK_OF_POS[c * 100: (c + 1) * 100]   # [100]
        k_valid = ktok_pos >= 0
        ktok = np.where(k_valid, ktok_pos, 0)
        blk = gathered[qtok[None, :], ktok[:, None], :]   # [100, P2, H]
        blk = blk.transpose(0, 2, 1)                      # [100, H, P2]
        blk = np.where(q_valid[None, None, :], blk, 0.0)
        blk = np.where(k_valid[:, None, None], blk, DUMMY_BIAS)
        relb4[:, :, c, :] = blk
    relb = np.exp(relb4.reshape(100, H * 2 * P2))

    # mixblk[wgi*12+h, wgi'*12+k] = d(wgi,wgi') mix[h,k]
    mixblk = np.kron(np.eye(10, dtype=np.float32), mix)
    bp_rep = np.broadcast_to(bp_eff.reshape(1, 768), (128, 768))

    common = {
        "wqk": _to_bf16(wqk),
        "wv": _to_bf16(wvt),
        "wp": _to_bf16(wpt),
        "relb": _to_bf16(relb),
        "mixblk": _to_bf16(mixblk),
        "bqk": bqk_sb,
        "bp": _to_bf16(bp_rep),
    }
    in_maps = []
    for ci in range(NCORES):
        xs = x[ci * BL: (ci + 1) * BL]              # [BL, N, C]
        xp = np.zeros((BL, P2, C), np.float32)
        xp[:, _POS_OF_TOK, :] = xs
        xt = xp.reshape(BL * P2, C).T               # [C, T2]
        # chunk-major xT: per chunk [128, 6*nsz], col = base6 + kt*nsz + n
        parts = []
        for (n0, nsz) in TOKC:
            blk = np.ascontiguousarray(xt[:, n0:n0 + nsz])
            parts.append(blk.reshape(KT, 128, nsz).transpose(1, 0, 2).reshape(128, KT * nsz))
        m = dict(common)
        m["xT"] = _to_bf16(np.concatenate(parts, axis=1))
        in_maps.append(m)
    return in_maps


def _get_compiled():
    if "nc" not in _COMPILED:
        _COMPILED["nc"] = _build_graph()
    return _COMPILED["nc"]


LAST_EXEC_NS = None
LAST_RESULTS = None


def _ensure_ntff_hook():
    """The agent image's antenv package lacks axon_hooks; synthesize it so
    run_bass_kernel_spmd(trace=True) can capture NTFF profiles."""
    import types

    if "antenv.axon_hooks" in sys.modules:
        return
    try:
        sys.path.insert(0, "/root/.axon_site")
        from trn_agent_boot.trn_boot import _ntff_profile_via_ctypes

        hook = _ntff_profile_via_ctypes("/opt/axon/libaxon_pjrt.so")
    except Exception:
        hook = None
    mod = types.ModuleType("antenv.axon_hooks")
    _state = {"hook": hook}
    mod.get_axon_ntff_profile_hook = lambda: _state["hook"]
    mod.set_axon_ntff_profile_hook = lambda h: _state.__setitem__("hook", h)
    sys.modules["antenv.axon_hooks"] = mod


def kernel(**inputs) -> np.ndarray:
    global LAST_EXEC_NS, LAST_RESULTS
    nc = _get_compiled()
    in_maps = _preprocess(inputs)
    from concourse.bass_utils import run_bass_kernel_spmd

    trace = os.environ.get("BASS_KERNEL_PROFILE", "0") == "1"
    if trace:
        _ensure_ntff_hook()
    res = run_bass_kernel_spmd(nc, in_maps, core_ids=list(range(NCORES)), trace=trace)
    LAST_EXEC_NS = res.exec_time_ns
    LAST_RESULTS = res
    outs = []
    for i in range(NCORES):
        o = np.asarray(res.results[i]["out"], dtype=np.float32).reshape(BL, P2, C)
        outs.append(o[:, _POS_OF_TOK, :])           # drop dummies, un-permute
    return np.concatenate(outs, axis=0).astype(np.float32)
